# revision 16
# baseline (speedup 1.0000x reference)
"""LoRA layer kernel for Trainium2 (Bass/Tile), data-parallel over 8 NeuronCores.

Math:  out = (x @ B) @ A * (32/16)   with x [4,2048,4096], B [4096,16], A [16,4096].

Strategy:
  - Flatten tokens (4*2048=8192), shard 1024 tokens per core (data parallel).
  - Host-side layout prep per shard: x is fed TRANSPOSED and partition-major
    (xT [ntb, split, 128, cpl, tb] fp16) so every DMA descriptor covers a
    long (8 KiB) contiguous DRAM run per partition.
  - All matmul operands are fp16 (full-rate PE streaming); PSUM stays f32.
  - ALL x loads are issued before any compute/store; loads and stores share
    the sync-engine HWDGE FIFO but every load precedes every store in
    program order, so prefetch is never blocked.
  - mm1: 4-way column-group packed (tile_position) chunk matmuls; a
    selector matmul per token subtile reduces the 4 column-group partials
    and lands them directly in row group 32*st (tile_position again), so
    one PSUM->SBUF copy feeds the row-packed mm2.
  - mm2: row-group packed across token subtiles; PSUM tiles span 2 banks
    ([128,1024]) so each PSUM->SBUF copy moves FD=1024 (amortizes the
    fixed per-copy overhead on DVE/ACT, which are the copy bottleneck).
    Copies split ~5:3 ACT:DVE (ACT is faster per copy and otherwise idle).
  - Software pipelining: block b's mm2/copies/stores are emitted AFTER
    block b+1's... actually BEFORE block b+1's mm1 in program order so the
    per-engine FIFOs never hold an instruction whose deps lag a ready one.
"""

import os
import numpy as np

IN = 4096
OUT = 4096
R = 16
N_CORES = 8
SCALE = 32.0 / 16.0
P = 128
NB = IN // P  # 32 contraction chunks


def _install_profile_hook():
    """Best-effort: register the axon NTFF profiling hook that this image's
    `antenv` package is missing, so run_bass_kernel_spmd(trace=True) can
    return exec_time_ns. Harmless no-op when anything is unavailable."""
    try:
        import sys
        import types

        if "antenv.axon_hooks" in sys.modules:
            return
        try:
            import antenv  # noqa: F401
        except ImportError:
            return
        mod = types.ModuleType("antenv.axon_hooks")
        mod._hook = None

        def set_axon_ntff_profile_hook(h):
            mod._hook = h

        def get_axon_ntff_profile_hook():
            return mod._hook

        mod.set_axon_ntff_profile_hook = set_axon_ntff_profile_hook
        mod.get_axon_ntff_profile_hook = get_axon_ntff_profile_hook
        sys.modules["antenv.axon_hooks"] = mod
        import antenv as _antenv

        _antenv.axon_hooks = mod

        so_path = "/opt/axon/libaxon_pjrt.so"
        if os.path.exists(so_path):
            try:
                from trn_agent_boot.trn_boot import _ntff_profile_via_ctypes

                hook = _ntff_profile_via_ctypes(so_path)
                if hook is not None:
                    mod._hook = hook
            except Exception:
                pass
    except Exception:
        pass


_install_profile_hook()

_NC_CACHE = {}


def build_nc(tok, tb=512, load_split=4):
    """Build + compile the per-core Bass program for `tok` tokens/core."""
    key = (tok, tb, load_split)
    if key in _NC_CACHE:
        return _NC_CACHE[key]

    import concourse.bacc as bacc
    import concourse.tile as tile
    from concourse import mybir

    f32 = mybir.dt.float32
    f16 = mybir.dt.float16
    tb = min(tb, tok)
    assert tok % tb == 0 and tb % P == 0
    ntb = tok // tb
    load_split = min(load_split, NB)
    cpl = NB // load_split  # chunks per load descriptor set

    nst = tb // P  # token subtiles per block (row groups for mm2)
    OC = 1024  # mm2 output-column chunk (PSUM tile spans 2 banks)

    nc = bacc.Bacc("TRN2", target_bir_lowering=False, debug=False)
    xT = nc.dram_tensor("xT", [ntb, load_split, P, cpl, tb], f16, kind="ExternalInput").ap()
    Bt = nc.dram_tensor("Bt", [P, NB, 2 * R], f16, kind="ExternalInput").ap()
    Ar = nc.dram_tensor("Ar", [R, OUT], f16, kind="ExternalInput").ap()
    Ss = nc.dram_tensor("Ss", [P, 2 * R], f16, kind="ExternalInput").ap()
    out = nc.dram_tensor("out", [tok, OUT], f16, kind="ExternalOutput").ap()

    with tile.TileContext(nc) as tc:
        with (
            tc.tile_pool(name="const", bufs=1) as const_pool,
            tc.tile_pool(name="xin", bufs=1) as x_pool,
            tc.tile_pool(name="xbt", bufs=2) as xbt_pool,
            tc.tile_pool(name="ps1", bufs=1, space="PSUM") as ps1,
            tc.tile_pool(name="psS", bufs=1, space="PSUM") as psS,
            tc.tile_pool(name="ps2", bufs=3, space="PSUM") as ps2,
            tc.tile_pool(name="osb", bufs=2) as out_pool,
        ):
            B_sb = const_pool.tile([P, NB, 2 * R], f16)
            nc.sync.dma_start(out=B_sb[:], in_=Bt[:])
            S_sb = const_pool.tile([P, 2 * R], f16)
            nc.sync.dma_start(out=S_sb[:], in_=Ss[:])
            A_sb = const_pool.tile([P, OUT], f16)

            # ALL x loads up front (keeps the sync HWDGE FIFO store-free
            # until every prefetch has been issued). A's replication DMAs
            # (needed first by mm2 of block 0) slot in after block 0's x.
            xT_sbs = []
            for tbi in range(ntb):
                xT_sb = x_pool.tile([P, NB, tb], f16, name=f"xT{tbi}")
                for li in range(load_split):
                    nc.sync.dma_start(
                        out=xT_sb[:, li * cpl : (li + 1) * cpl, :],
                        in_=xT[tbi, li],
                    )
                xT_sbs.append(xT_sb)
                if tbi == 0:
                    # A replicated on-device into nst row groups: rows
                    # 32g+r hold A_scaled[r, :]
                    for st in range(nst):
                        nc.sync.dma_start(
                            out=A_sb[32 * st : 32 * st + R, :], in_=Ar[:]
                        )

            cp = 0  # ACT/DVE copy scheduling counter

            def front_half(tbi):
                """mm1 + selector + xbt for block tbi."""
                xT_sb = xT_sbs[tbi]
                # mm1, 4-way column-group packed: col group g accumulates
                # chunks {4k+g} into PSUM partitions [32g, 32g+16)
                ps_part = ps1.tile([P, tb], f32)
                for c8 in range(NB // 4):
                    for g in range(4):
                        c = c8 * 4 + g
                        nc.tensor.matmul(
                            ps_part[32 * g : 32 * g + 2 * R, :],
                            lhsT=B_sb[:, c, :],
                            rhs=xT_sb[:, c, :],
                            start=(c8 == 0),
                            stop=(c8 == NB // 4 - 1),
                            tile_position=(0, 32 * g),
                            skip_group_check=True,
                        )
                part_sb = xbt_pool.tile([P, tb], f16, tag="part")
                nc.vector.tensor_copy(part_sb[:], ps_part[:])
                # selector matmuls: reduce the 4 col-group partials to a
                # [16, 128] xbT per subtile, landed directly in row group
                # 32*st via tile_position so one copy serves the packed mm2
                ps_sel = psS.tile([P, P], f32)
                for st in range(nst):
                    nc.tensor.matmul(
                        ps_sel[32 * st : 32 * st + 2 * R, :],
                        lhsT=S_sb[:],
                        rhs=part_sb[:, st * P : (st + 1) * P],
                        start=True,
                        stop=True,
                        tile_position=(0, 32 * st),
                        skip_group_check=True,
                    )
                xbt_sb = xbt_pool.tile([P, P], f16, tag="xbt")
                nc.vector.tensor_copy(xbt_sb[: 32 * nst, :], ps_sel[: 32 * nst, :])
                o_sbs = [
                    out_pool.tile([P, OUT], f16, name=f"osb{st}_{tbi}", tag=f"osb{st}")
                    for st in range(nst)
                ]
                return (tbi, xbt_sb, o_sbs)

            NOP = OUT // OC

            def back_half(state, op_lo, op_hi):
                """mm2 + PSUM->SBUF copies for block tbi, output-column
                chunks [op_lo, op_hi); stores once the last chunk is done."""
                nonlocal cp
                tbi, xbt_sb, o_sbs = state
                for op in range(op_lo, op_hi):
                    for st in range(nst):
                        ps_o = ps2.tile([P, OC], f32)
                        for h in range(OC // 512):
                            o0 = op * OC + h * 512
                            nc.tensor.matmul(
                                ps_o[:, h * 512 : (h + 1) * 512],
                                lhsT=xbt_sb[32 * st : 32 * st + R, :],
                                rhs=A_sb[32 * st : 32 * st + R, o0 : o0 + 512],
                                start=True,
                                stop=True,
                                tile_position=(32 * st, 0),
                                skip_group_check=True,
                            )
                        # split PSUM->SBUF copies ~5:3 ACT:DVE (ACT is the
                        # faster copier; DVE also does the part/xbt copies)
                        if cp % 8 in (1, 3, 5):
                            nc.vector.tensor_copy(
                                o_sbs[st][:, op * OC : (op + 1) * OC], ps_o[:]
                            )
                        else:
                            nc.scalar.activation(
                                o_sbs[st][:, op * OC : (op + 1) * OC],
                                ps_o[:],
                                mybir.ActivationFunctionType.Copy,
                            )
                        cp += 1
                if op_hi == NOP:
                    for st in range(nst):
                        t0 = tbi * tb + st * P
                        nc.sync.dma_start(out=out[t0 : t0 + P, :], in_=o_sbs[st][:])

            # software pipeline with sub-block interleaving: the first half
            # of block b-1's mm2 is emitted before front_half(b), the rest
            # after, so no engine FIFO holds a long run of instructions
            # whose dependencies lag another ready instruction
            prev = None
            for tbi in range(ntb):
                if prev is not None:
                    back_half(prev, 0, NOP // 2)
                cur = front_half(tbi)
                if prev is not None:
                    back_half(prev, NOP // 2, NOP)
                prev = cur
            back_half(prev, 0, NOP)

    nc.compile()
    _NC_CACHE[key] = nc
    return nc


TB = 512
LOAD_SPLIT = 4


def make_in_maps(x, lora_A, lora_B, n_cores=N_CORES):
    x = np.asarray(x, dtype=np.float32)
    A = np.asarray(lora_A, dtype=np.float32)
    B = np.asarray(lora_B, dtype=np.float32)
    xf = x.reshape(-1, IN)
    ntok = xf.shape[0] // n_cores
    tb = min(TB, ntok)
    ntb = ntok // tb
    split = min(LOAD_SPLIT, NB)
    cpl = NB // split
    A_scaled = np.ascontiguousarray(A * np.float32(SCALE), dtype=np.float16)
    S_sel = np.zeros((P, 2 * R), dtype=np.float16)
    for g in range(4):
        S_sel[32 * g : 32 * g + R, :R] = np.eye(R, dtype=np.float16)
    B_resh = np.zeros((P, NB, 2 * R), dtype=np.float16)
    B_resh[:, :, :R] = B.reshape(NB, P, R).transpose(1, 0, 2)
    in_maps = []
    for c in range(n_cores):
        shard = xf[c * ntok : (c + 1) * ntok]
        # partition-major pre-tile: [ntb, split, P, cpl, tb]
        # xt[tbi, li, p, cl, t] = shard[tbi*tb + t, (li*cpl + cl)*128 + p]
        xt = np.ascontiguousarray(
            shard.reshape(ntb, tb, split, cpl, P).transpose(0, 2, 4, 3, 1),
            dtype=np.float16,
        )
        in_maps.append(
            {
                "xT": xt,
                "Bt": B_resh,
                "Ar": A_scaled,
                "Ss": S_sel,
            }
        )
    return in_maps, ntok


def kernel_with_results(x, lora_A, lora_B, trace=False, **kwargs):
    from concourse.bass_utils import run_bass_kernel_spmd

    in_maps, ntok = make_in_maps(x, lora_A, lora_B)
    nc = build_nc(ntok, tb=TB, load_split=LOAD_SPLIT)
    res = run_bass_kernel_spmd(nc, in_maps, list(range(N_CORES)), trace=trace, **kwargs)
    out = np.concatenate([r["out"] for r in res.results], axis=0).astype(np.float32)
    return out.reshape(np.asarray(x).shape[:-1] + (OUT,)), res


def kernel(x, lora_A, lora_B):
    out, _ = kernel_with_results(x, lora_A, lora_B)
    return out


# revision 19
# speedup vs baseline: 1.0332x; 1.0332x over previous
"""LoRA layer kernel for Trainium2 (Bass/Tile), data-parallel over 8 NeuronCores.

Math:  out = (x @ B) @ A * (32/16)   with x [4,2048,4096], B [4096,16], A [16,4096].

Strategy:
  - Flatten tokens (4*2048=8192), shard 1024 tokens per core (data parallel).
  - Host-side layout prep per shard: x is fed TRANSPOSED and partition-major
    (xT [ntb, split, 128, cpl, tb] fp16) so every DMA descriptor covers a
    long (8 KiB) contiguous DRAM run per partition.
  - All matmul operands are fp16 (full-rate PE streaming); PSUM stays f32.
  - ALL x loads are issued before any compute/store; loads and stores share
    the sync-engine HWDGE FIFO but every load precedes every store in
    program order, so prefetch is never blocked.
  - mm1: 4-way column-group packed (tile_position) chunk matmuls; a
    selector matmul per token subtile reduces the 4 column-group partials
    and lands them directly in row group 32*st (tile_position again), so
    one PSUM->SBUF copy feeds the row-packed mm2.
  - mm2: row-group packed across token subtiles; PSUM tiles span 2 banks
    ([128,1024]) so each PSUM->SBUF copy moves FD=1024 (amortizes the
    fixed per-copy overhead on DVE/ACT, which are the copy bottleneck).
    Copies split ~5:3 ACT:DVE (ACT is faster per copy and otherwise idle).
  - Software pipelining: block b's mm2/copies/stores are emitted AFTER
    block b+1's... actually BEFORE block b+1's mm1 in program order so the
    per-engine FIFOs never hold an instruction whose deps lag a ready one.
"""

import os
import numpy as np

IN = 4096
OUT = 4096
R = 16
N_CORES = 8
SCALE = 32.0 / 16.0
P = 128
NB = IN // P  # 32 contraction chunks


def _install_profile_hook():
    """Best-effort: register the axon NTFF profiling hook that this image's
    `antenv` package is missing, so run_bass_kernel_spmd(trace=True) can
    return exec_time_ns. Harmless no-op when anything is unavailable."""
    try:
        import sys
        import types

        if "antenv.axon_hooks" in sys.modules:
            return
        try:
            import antenv  # noqa: F401
        except ImportError:
            return
        mod = types.ModuleType("antenv.axon_hooks")
        mod._hook = None

        def set_axon_ntff_profile_hook(h):
            mod._hook = h

        def get_axon_ntff_profile_hook():
            return mod._hook

        mod.set_axon_ntff_profile_hook = set_axon_ntff_profile_hook
        mod.get_axon_ntff_profile_hook = get_axon_ntff_profile_hook
        sys.modules["antenv.axon_hooks"] = mod
        import antenv as _antenv

        _antenv.axon_hooks = mod

        so_path = "/opt/axon/libaxon_pjrt.so"
        if os.path.exists(so_path):
            try:
                from trn_agent_boot.trn_boot import _ntff_profile_via_ctypes

                hook = _ntff_profile_via_ctypes(so_path)
                if hook is not None:
                    mod._hook = hook
            except Exception:
                pass
    except Exception:
        pass


_install_profile_hook()

_NC_CACHE = {}


def build_nc(tok, tb=512, load_split=4):
    """Build + compile the per-core Bass program for `tok` tokens/core."""
    key = (tok, tb, load_split)
    if key in _NC_CACHE:
        return _NC_CACHE[key]

    import concourse.bacc as bacc
    import concourse.tile as tile
    from concourse import mybir

    f32 = mybir.dt.float32
    f16 = mybir.dt.float16
    tb = min(tb, tok)
    assert tok % tb == 0 and tb % P == 0
    ntb = tok // tb
    load_split = min(load_split, NB)
    cpl = NB // load_split  # chunks per load descriptor set

    nst = tb // P  # token subtiles per block (row groups for mm2)
    OC = 1024  # mm2 output-column chunk (PSUM tile spans 2 banks)

    nc = bacc.Bacc("TRN2", target_bir_lowering=False, debug=False)
    xT = nc.dram_tensor("xT", [ntb, load_split, P, cpl, tb], f16, kind="ExternalInput").ap()
    Bt = nc.dram_tensor("Bt", [P, NB, 2 * R], f16, kind="ExternalInput").ap()
    Ar = nc.dram_tensor("Ar", [R, OUT], f16, kind="ExternalInput").ap()
    Ss = nc.dram_tensor("Ss", [P, 2 * R], f16, kind="ExternalInput").ap()
    out = nc.dram_tensor("out", [tok, OUT], f16, kind="ExternalOutput").ap()

    with tile.TileContext(nc) as tc:
        with (
            tc.tile_pool(name="const", bufs=1) as const_pool,
            tc.tile_pool(name="xin", bufs=1) as x_pool,
            tc.tile_pool(name="xbt", bufs=2) as xbt_pool,
            tc.tile_pool(name="ps1", bufs=1, space="PSUM") as ps1,
            tc.tile_pool(name="psS", bufs=1, space="PSUM") as psS,
            tc.tile_pool(name="ps2", bufs=3, space="PSUM") as ps2,
            tc.tile_pool(name="osb", bufs=2) as out_pool,
        ):
            # consts go on the SCALAR HWDGE queue so the sync queue carries
            # a clean, uninterrupted x-load stream (the A loads' 16-partition
            # descriptors otherwise stall the ring for ~4us mid-stream)
            B_sb = const_pool.tile([P, NB, 2 * R], f16)
            nc.scalar.dma_start(out=B_sb[:], in_=Bt[:])
            S_sb = const_pool.tile([P, 2 * R], f16)
            nc.scalar.dma_start(out=S_sb[:], in_=Ss[:])
            # A loaded ONCE into row group 0, replicated to the other row
            # groups on-chip (DVE partition-shift copies, off the DMA pipe)
            A_sb = const_pool.tile([P, OUT], f16)
            nc.scalar.dma_start(out=A_sb[:R, :], in_=Ar[:])
            for st in range(1, nst):
                nc.vector.tensor_copy(A_sb[32 * st : 32 * st + R, :], A_sb[:R, :])

            # ALL x loads up front (keeps the sync HWDGE FIFO store-free
            # until every prefetch has been issued)
            xT_sbs = []
            for tbi in range(ntb):
                xT_sb = x_pool.tile([P, NB, tb], f16, name=f"xT{tbi}")
                for li in range(load_split):
                    nc.sync.dma_start(
                        out=xT_sb[:, li * cpl : (li + 1) * cpl, :],
                        in_=xT[tbi, li],
                    )
                xT_sbs.append(xT_sb)

            cp = 0  # ACT/DVE copy scheduling counter

            def front_half(tbi):
                """mm1 + selector + xbt for block tbi."""
                xT_sb = xT_sbs[tbi]
                # mm1, 4-way column-group packed: col group g accumulates
                # chunks {4k+g} into PSUM partitions [32g, 32g+16)
                ps_part = ps1.tile([P, tb], f32)
                for c8 in range(NB // 4):
                    for g in range(4):
                        c = c8 * 4 + g
                        nc.tensor.matmul(
                            ps_part[32 * g : 32 * g + 2 * R, :],
                            lhsT=B_sb[:, c, :],
                            rhs=xT_sb[:, c, :],
                            start=(c8 == 0),
                            stop=(c8 == NB // 4 - 1),
                            tile_position=(0, 32 * g),
                            skip_group_check=True,
                        )
                part_sb = xbt_pool.tile([P, tb], f16, tag="part")
                nc.vector.tensor_copy(part_sb[:], ps_part[:])
                # selector matmuls: reduce the 4 col-group partials to a
                # [16, 128] xbT per subtile, landed directly in row group
                # 32*st via tile_position so one copy serves the packed mm2
                ps_sel = psS.tile([P, P], f32)
                for st in range(nst):
                    nc.tensor.matmul(
                        ps_sel[32 * st : 32 * st + 2 * R, :],
                        lhsT=S_sb[:],
                        rhs=part_sb[:, st * P : (st + 1) * P],
                        start=True,
                        stop=True,
                        tile_position=(0, 32 * st),
                        skip_group_check=True,
                    )
                xbt_sb = xbt_pool.tile([P, P], f16, tag="xbt")
                nc.vector.tensor_copy(xbt_sb[: 32 * nst, :], ps_sel[: 32 * nst, :])
                o_sbs = [
                    out_pool.tile([P, OUT], f16, name=f"osb{st}_{tbi}", tag=f"osb{st}")
                    for st in range(nst)
                ]
                return (tbi, xbt_sb, o_sbs)

            NOP = OUT // OC

            def back_half(state, op_lo, op_hi):
                """mm2 + PSUM->SBUF copies for block tbi, output-column
                chunks [op_lo, op_hi); stores once the last chunk is done."""
                nonlocal cp
                tbi, xbt_sb, o_sbs = state
                for op in range(op_lo, op_hi):
                    for st in range(nst):
                        ps_o = ps2.tile([P, OC], f32)
                        for h in range(OC // 512):
                            o0 = op * OC + h * 512
                            nc.tensor.matmul(
                                ps_o[:, h * 512 : (h + 1) * 512],
                                lhsT=xbt_sb[32 * st : 32 * st + R, :],
                                rhs=A_sb[32 * st : 32 * st + R, o0 : o0 + 512],
                                start=True,
                                stop=True,
                                tile_position=(32 * st, 0),
                                skip_group_check=True,
                            )
                        # split PSUM->SBUF copies ~5:3 ACT:DVE (ACT is the
                        # faster copier; DVE also does the part/xbt copies)
                        if cp % 8 in (1, 3, 5):
                            nc.vector.tensor_copy(
                                o_sbs[st][:, op * OC : (op + 1) * OC], ps_o[:]
                            )
                        else:
                            nc.scalar.activation(
                                o_sbs[st][:, op * OC : (op + 1) * OC],
                                ps_o[:],
                                mybir.ActivationFunctionType.Copy,
                            )
                        cp += 1
                    # release stores at half-subtile granularity so the
                    # store stream starts mid-block and the tail shrinks
                    if op == NOP // 2 - 1 or op == NOP - 1:
                        c0 = 0 if op == NOP // 2 - 1 else (NOP // 2) * OC
                        c1 = (op + 1) * OC
                        for st2 in range(nst):
                            t0 = tbi * tb + st2 * P
                            nc.sync.dma_start(
                                out=out[t0 : t0 + P, c0:c1],
                                in_=o_sbs[st2][:, c0:c1],
                            )

            # block b's back half is emitted before block b+1's front: with
            # clean load streaming, mm1(b+1) is load-paced anyway, and this
            # keeps mm2(b) off the tensor FIFO's critical path
            for tbi in range(ntb):
                back_half(front_half(tbi), 0, NOP)

    nc.compile()
    _NC_CACHE[key] = nc
    return nc


TB = 512
LOAD_SPLIT = 4


def make_in_maps(x, lora_A, lora_B, n_cores=N_CORES):
    x = np.asarray(x, dtype=np.float32)
    A = np.asarray(lora_A, dtype=np.float32)
    B = np.asarray(lora_B, dtype=np.float32)
    xf = x.reshape(-1, IN)
    ntok = xf.shape[0] // n_cores
    tb = min(TB, ntok)
    ntb = ntok // tb
    split = min(LOAD_SPLIT, NB)
    cpl = NB // split
    A_scaled = np.ascontiguousarray(A * np.float32(SCALE), dtype=np.float16)
    S_sel = np.zeros((P, 2 * R), dtype=np.float16)
    for g in range(4):
        S_sel[32 * g : 32 * g + R, :R] = np.eye(R, dtype=np.float16)
    B_resh = np.zeros((P, NB, 2 * R), dtype=np.float16)
    B_resh[:, :, :R] = B.reshape(NB, P, R).transpose(1, 0, 2)
    in_maps = []
    for c in range(n_cores):
        shard = xf[c * ntok : (c + 1) * ntok]
        # partition-major pre-tile: [ntb, split, P, cpl, tb]
        # xt[tbi, li, p, cl, t] = shard[tbi*tb + t, (li*cpl + cl)*128 + p]
        xt = np.ascontiguousarray(
            shard.reshape(ntb, tb, split, cpl, P).transpose(0, 2, 4, 3, 1),
            dtype=np.float16,
        )
        in_maps.append(
            {
                "xT": xt,
                "Bt": B_resh,
                "Ar": A_scaled,
                "Ss": S_sel,
            }
        )
    return in_maps, ntok


def kernel_with_results(x, lora_A, lora_B, trace=False, **kwargs):
    from concourse.bass_utils import run_bass_kernel_spmd

    in_maps, ntok = make_in_maps(x, lora_A, lora_B)
    nc = build_nc(ntok, tb=TB, load_split=LOAD_SPLIT)
    res = run_bass_kernel_spmd(nc, in_maps, list(range(N_CORES)), trace=trace, **kwargs)
    out = np.concatenate([r["out"] for r in res.results], axis=0).astype(np.float32)
    return out.reshape(np.asarray(x).shape[:-1] + (OUT,)), res


def kernel(x, lora_A, lora_B):
    out, _ = kernel_with_results(x, lora_A, lora_B)
    return out
